# revision 8
# baseline (speedup 1.0000x reference)
"""Causal self-attention, tensor-parallel over heads across 8 NeuronCores.

Reference:  qkv = x @ w_qkv + b_qkv;  per-head causal softmax attention;
            out = y @ w_proj + b_proj.
Shapes: x [2, 2048, 1024], H=16 heads, head_dim 64.

Sharding (per core c of 8): heads {2c, 2c+1}.
  - w_qkv columns for q/k/v of those heads  -> [1024, 384]
  - w_proj rows for those heads             -> [128, 1024]
  - each core computes a partial projection output; host sums the 8
    partials (the "all-reduce after proj").

v2 design (bf16 datapath, engine-balanced):
  - All DRAM I/O in bf16 (halves DMA traffic; rel-err budget 2e-2 vs
    ~1e-3 realized). PSUM accumulation stays fp32.
  - Bias algebra done host-side where exact: k-bias drops out of
    softmax (per-row constant shift); v-bias folds into b_proj
    (softmax weights sum to 1 -> out += b_v @ w_proj); only the q-bias
    runs on device, fused (with the 1/sqrt(HD) scale) into the ACT
    PSUM->SBUF move of Q.
  - V is computed directly in [t, feature] orientation (stationary = x
    tile, moving = w_v): no PE transposes; all 4 t-tiles of a chunk
    accumulate into one PSUM bank, copied out with 2 wide Pool copies.
  - bf16 matmuls run full rate at any width, so causal widths are
    exact 128-multiples.
  - ACT per-op overhead amortized: same-width score tiles are computed
    in [128, 2, 512] two-bank PSUM tiles and exp'd in ONE activation
    over 1024 columns; Q and K share one such pair tile per chunk.
  - Engine placement: PE matmuls; ACT exp + Q bias-move; DVE diag
    masks, reciprocal, normalize-mul, half the proj copies; Pool the
    other copies + reciprocal partition-broadcast; SP queue loads,
    ACT queue stores.
  - Tail: batch-1 attention q-chunks run descending so the drain ends
    on the smallest group.
"""

import numpy as np
import ml_dtypes

import concourse.bacc as bacc
import concourse.mybir as mybir
import concourse.tile as tile
from concourse import bass_utils

# Problem shapes (hardcoded per contest contract)
B, T, D = 2, 2048, 1024
H, HD = 16, 64
N_CORES = 8
HLOC = H // N_CORES      # 2 heads per core
FQ = HLOC * HD           # 128 features per core per q/k/v
BT = B * T               # 4096
TQ = 512                 # q-chunk (matmul moving dim)
NQC = T // TQ            # 4 q-chunks per batch
NKT = T // 128           # 16 k-tiles per batch
NCT = D // 128           # 8 contraction tiles for qkv

F32 = mybir.dt.float32
BF16 = mybir.dt.bfloat16
EXP = mybir.ActivationFunctionType.Exp
IDENT = mybir.ActivationFunctionType.Identity


def build_nc(reps=1):
    nc = bacc.Bacc("TRN2", debug=False)

    xT = nc.dram_tensor("xT", (D, BT), BF16, kind="ExternalInput")
    wqkv = nc.dram_tensor("wqkv", (D, 3 * FQ), BF16, kind="ExternalInput")
    bq_d = nc.dram_tensor("bq", (128, 1), F32, kind="ExternalInput")
    wproj = nc.dram_tensor("wproj", (FQ, D), BF16, kind="ExternalInput")
    tri_d = nc.dram_tensor("tri", (128, 128), BF16, kind="ExternalInput")
    out = nc.dram_tensor("out", (BT, D), BF16, kind="ExternalOutput")

    xT_r = xT.rearrange("(ct p) t -> p ct t", p=128)
    wq_r = wqkv.rearrange("(ct p) (f m) -> p f ct m", p=128, f=3)

    with tile.TileContext(nc) as tc:
        with (
            tc.tile_pool(name="const", bufs=1) as cpool,
            tc.tile_pool(name="xt", bufs=3) as xpool,
            tc.tile_pool(name="pp", bufs=1) as ppool,
            tc.tile_pool(name="sm", bufs=2) as spool,
            tc.tile_pool(name="osb", bufs=4) as opool,
            tc.tile_pool(name="ps", bufs=1, space="PSUM") as ps,
        ):
            # ---- persistent tiles; first-needed DMAs first ----
            wsb = cpool.tile([128, 3, NCT, 128], BF16)
            nc.sync.dma_start(wsb[:, 0], wq_r[:, 0])
            bqsb = cpool.tile([128, 1], F32)
            wpsb = cpool.tile([128, D], BF16)
            trisb = cpool.tile([128, 128], BF16)

            qT = cpool.tile([128, BT], BF16)
            kTt = cpool.tile([128, BT], BF16)
            yT = cpool.tile([128, BT], BF16)
            Vp = cpool.tile([128, HLOC, B * NKT, HD + 1], BF16)
            # V' ones-column (sumexp trick), one strided memset
            nc.gpsimd.memset(Vp[:, :, :, HD:HD + 1], 1.0)

            pend = None

            def emit_proj(qoff_abs):
                for tt in range(TQ // 128):
                    toff = qoff_abs + tt * 128
                    osb = opool.tile([128, D], BF16, tag="osb")
                    for e in range(D // TQ):
                        ppj = ps.tile([128, TQ], F32, tag="ss", bufs=2)
                        nc.tensor.matmul(
                            ppj[:], yT[:, toff:toff + 128],
                            wpsb[:, e * TQ:(e + 1) * TQ],
                            start=True, stop=True)
                        nc.vector.tensor_copy(
                            osb[:, e * TQ:(e + 1) * TQ], ppj[:])
                    nc.scalar.dma_start(out[toff:toff + 128, :], osb[:])

            def emit_qkv(b, tcx, first=False, very_first=False):
                off = b * T + tcx * TQ
                xt = xpool.tile([128, NCT, TQ], BF16, name="xt")
                if very_first:
                    # first matmul gates only on ct-tile 0
                    nc.sync.dma_start(xt[:, 0:1], xT_r[:, 0:1, off:off + TQ])
                    nc.sync.dma_start(xt[:, 1:], xT_r[:, 1:, off:off + TQ])
                else:
                    nc.sync.dma_start(xt[:, 0:NCT // 2],
                                      xT_r[:, 0:NCT // 2, off:off + TQ])
                    nc.sync.dma_start(xt[:, NCT // 2:],
                                      xT_r[:, NCT // 2:, off:off + TQ])
                if first:
                    # later-phase constants ride behind the first x chunk
                    nc.sync.dma_start(bqsb[:], bq_d[:])
                    nc.sync.dma_start(wsb[:, 1], wq_r[:, 1])
                    nc.sync.dma_start(wsb[:, 2], wq_r[:, 2])
                    nc.sync.dma_start(trisb[:], tri_d[:])
                    nc.sync.dma_start(wpsb[:], wproj[:])
                # Q and K accumulate into one two-bank pair tile.
                qk = ps.tile([128, 2, TQ], F32, tag="sp", bufs=2, name="qk")
                for f in range(2):
                    for ct in range(NCT):
                        nc.tensor.matmul(
                            qk[:, f], wsb[:, f, ct, :], xt[:, ct, :],
                            start=(ct == 0), stop=(ct == NCT - 1),
                        )
                # Q: 1/sqrt(HD) scale + (pre-scaled) bias fused in the ACT
                # move; K: plain Pool copy (k-bias drops out of softmax)
                nc.scalar.activation(
                    qT[:, off:off + TQ], qk[:, 0], IDENT,
                    bias=bqsb[:, 0:1], scale=1.0 / np.sqrt(HD))
                nc.scalar.activation(
                    kTt[:, off:off + TQ], qk[:, 1],
                    mybir.ActivationFunctionType.Copy)
                # V in [t, feature] orientation; 4 t-tiles share one bank
                vq = ps.tile([128, TQ], F32, tag="ss", bufs=2, name="vq")
                for j in range(TQ // 128):
                    for ct in range(NCT):
                        nc.tensor.matmul(
                            vq[:, j * 128:(j + 1) * 128],
                            xt[:, ct, j * 128:(j + 1) * 128],
                            wsb[:, 2, ct, :],
                            start=(ct == 0), stop=(ct == NCT - 1),
                        )
                kti0 = b * NKT + tcx * (TQ // 128)
                vqr = vq[:].rearrange("p (j f) -> p j f", j=TQ // 128)
                for h in range(HLOC):
                    nc.vector.tensor_copy(
                        Vp[:, h, kti0:kti0 + TQ // 128, 0:HD],
                        vqr[:, :, h * HD:(h + 1) * HD])

            def norm_stage1(pend):
                # reciprocal of the sumexp row, broadcast to HD partitions
                rec = spool.tile([1, TQ], F32, tag="rec", name="rec")
                nc.vector.reciprocal(rec[:], pend["po"][HD:HD + 1, :])
                rb = spool.tile([HD, TQ], F32, tag="rb", name="rb")
                nc.gpsimd.partition_broadcast(rb[:], rec[0:1, :])
                return rb

            def norm_stage2(pend, rb):
                nc.vector.tensor_mul(
                    yT[pend["hp"]:pend["hp"] + HD,
                       pend["qoff"]:pend["qoff"] + TQ],
                    pend["po"][0:HD, :], rb[:])
                if pend["last_head"]:
                    emit_proj(pend["qoff"])

            def emit_attn(b, qcx, h):
                nonlocal pend
                base = b * T
                qoff = base + qcx * TQ
                nkt_eff = (TQ // 128) * (qcx + 1)
                hp = HD * h
                diag0 = (TQ // 128) * qcx
                pps = {}

                def s_matmul(dst, kt, cs):
                    nc.tensor.matmul(
                        dst,
                        kTt[hp:hp + HD,
                            base + kt * 128:base + (kt + 1) * 128],
                        qT[hp:hp + HD, qoff + cs:qoff + TQ],
                        start=True, stop=True,
                    )

                # full-width tiles (all non-diag + diag r=0): exp in pairs
                fw = list(range(diag0 + 1))
                i = 0
                while i < len(fw):
                    if i + 1 < len(fw):
                        kta, ktb = fw[i], fw[i + 1]
                        sp = ps.tile([128, 2, TQ], F32, tag="sp", bufs=2,
                                     name="sp")
                        s_matmul(sp[:, 0], kta, 0)
                        s_matmul(sp[:, 1], ktb, 0)
                        pp2 = ppool.tile([128, 2, TQ], BF16, tag="pp2",
                                         bufs=8, name="pp2")
                        nc.scalar.activation(pp2[:], sp[:], EXP)
                        pps[kta] = (pp2[:, 0], 0)
                        pps[ktb] = (pp2[:, 1], 0)
                        i += 2
                    else:
                        kt = fw[i]
                        s1 = ps.tile([128, TQ], F32, tag="ss", bufs=2,
                                     name="s1")
                        s_matmul(s1[:], kt, 0)
                        pp1 = ppool.tile([128, TQ], BF16, tag="pp1",
                                         bufs=6, name="pp1")
                        nc.scalar.activation(pp1[:], s1[:], EXP)
                        pps[kt] = (pp1[:], 0)
                        i += 1
                # remaining diagonal tiles (width < 512)
                for kt in range(diag0 + 1, nkt_eff):
                    cs = 128 * (kt - diag0)
                    s1 = ps.tile([128, TQ], F32, tag="ss", bufs=2, name="s1")
                    s_matmul(s1[:, cs:TQ], kt, cs)
                    pp1 = ppool.tile([128, TQ], BF16, tag="pp1", bufs=6,
                                     name="pp1")
                    nc.scalar.activation(pp1[:, cs:TQ], s1[:, cs:TQ], EXP)
                    pps[kt] = (pp1[:], cs)
                # previous group's normalization, interleaved for overlap
                rb = norm_stage1(pend) if pend is not None else None
                # diagonal masks (all-SBUF -> legal + cheap on Pool)
                for kt in range(diag0, nkt_eff):
                    pp, cs = pps[kt]
                    nc.gpsimd.tensor_mul(
                        pp[:, cs:cs + 128], pp[:, cs:cs + 128], trisb[:])
                if pend is not None:
                    norm_stage2(pend, rb)
                # PV accumulation (+ sumexp via the ones column)
                po = ps.tile([128, TQ], F32, tag="o", bufs=2, name="po")
                for i, kt in enumerate(range(nkt_eff)):
                    pp, cs = pps[kt]
                    nc.tensor.matmul(
                        po[0:HD + 1, cs:TQ],
                        Vp[:, h, b * NKT + kt, :],
                        pp[:, cs:TQ],
                        start=(i == 0), stop=(i == nkt_eff - 1),
                    )
                pend = {"po": po, "hp": hp, "qoff": qoff,
                        "last_head": h == HLOC - 1}

            for _rep in range(reps):
                for tcx in range(NQC):
                    emit_qkv(0, tcx, first=(_rep == 0 and tcx == 0),
                             very_first=(_rep == 0 and tcx == 0))
                for qcx in range(NQC):
                    emit_attn(0, qcx, 0)
                    emit_attn(0, qcx, 1)
                    # batch-1 QKV rides along batch-0 attention, descending
                    # so the first batch-1 attn group's inputs land first
                    emit_qkv(1, NQC - 1 - qcx)
                for qcx in range(NQC - 1, -1, -1):
                    emit_attn(1, qcx, 0)
                    emit_attn(1, qcx, 1)

            # flush the last group's norm + projection
            if pend is not None:
                rb = norm_stage1(pend)
                norm_stage2(pend, rb)

    nc.finalize()
    return nc


def _make_tri():
    # tri[p, j] = 1.0 if j >= p (keep at-or-below diagonal of s_T[k, q])
    j = np.arange(128)[None, :]
    p = np.arange(128)[:, None]
    return (j >= p).astype(ml_dtypes.bfloat16)


_NC_CACHE = None
_LAST_IN_MAPS = None


def kernel(x, w_qkv, b_qkv, w_proj, b_proj):
    global _NC_CACHE, _LAST_IN_MAPS
    if _NC_CACHE is None:
        _NC_CACHE = build_nc()
    nc = _NC_CACHE

    x = np.asarray(x, dtype=np.float32)
    w_qkv = np.asarray(w_qkv, dtype=np.float32)
    b_qkv = np.asarray(b_qkv, dtype=np.float32)
    w_proj = np.asarray(w_proj, dtype=np.float32)
    b_proj = np.asarray(b_proj, dtype=np.float32)

    xT = np.ascontiguousarray(x.reshape(BT, D).T).astype(ml_dtypes.bfloat16)
    tri = _make_tri()

    in_maps = []
    for c in range(N_CORES):
        cols = slice(FQ * c, FQ * (c + 1))
        wq = np.concatenate(
            [w_qkv[:, cols], w_qkv[:, D:][:, cols], w_qkv[:, 2 * D:][:, cols]],
            axis=1).astype(ml_dtypes.bfloat16)       # [D, 384]
        # q-bias pre-scaled by 1/sqrt(HD) (fused with the ACT scale)
        bq = (b_qkv[cols] / np.sqrt(HD)).reshape(128, 1).astype(np.float32)
        in_maps.append({
            "xT": xT,
            "wqkv": np.ascontiguousarray(wq),
            "bq": bq,
            "wproj": np.ascontiguousarray(
                w_proj[cols, :]).astype(ml_dtypes.bfloat16),
            "tri": tri,
        })

    _LAST_IN_MAPS = in_maps
    res = bass_utils.run_bass_kernel_spmd(
        nc, in_maps, core_ids=list(range(N_CORES)))
    acc = res.results[0]["out"].astype(np.float32).copy()
    for c in range(1, N_CORES):
        acc += res.results[c]["out"].astype(np.float32)
    # exact host-side bias folds: +b_proj, and v-bias -> +b_v @ w_proj
    acc += (b_proj + b_qkv[2 * D:] @ w_proj)[None, :]
    return acc.reshape(B, T, D)
